# revision 1
# baseline (speedup 1.0000x reference)
"""ColAttention TRN2 kernel v2: 8-core data-parallel over batch (2 batches/core).

Math (per batch b, width-column w):
  Q = Wq@x+bq; K = Wk@x+bk; V = Wv@x+bv        (1x1 convs over c)
  S[h,g] = sum_q Q[q,h]K[q,g]; attn = softmax_g(S)
  out = gamma * (attn @ V^T)^T + x

Host folds bv via e = gamma*(I+gamma*Wv)^-1 bv: xb = x+e, bq' = bq-Wq@e,
bk' = bk-Wk@e => device never touches bv; residual add of xb is exact.

v2 design (vs v1): transposed-output scheme.
  - Per 4-column chunk: Q/K projections (N=384), S^T per column,
    exp on ACT, V^T per column, then U^T[h,c] = es_col^T-contracted
    matmul with M=h so the softmax normalizer r[h]=gamma/colsum lands
    on PARTITIONS -> fused (u*r)+xT in ONE scalar_tensor_tensor op.
  - colsum via N=1 matmul reusing es_col as lhsT (no PE broadcast, no
    normalize-multiply pass over es or U).
  - Output written bf16 into the xT tile in place, DMA'd per half,
    upcast to f32 on host. Input DMA'd in two layouts (c-major + h-major).
  - Single flat software-pipelined chunk loop (depth 2) keeps every
    engine continuously fed; all DMA overlapped.
"""
import sys

sys.path.insert(0, "/opt/trn_rl_repo")

import numpy as np
import ml_dtypes

import concourse.bass as bass
import concourse.bacc as bacc
import concourse.mybir as mybir
import concourse.tile as tile
from concourse.bass_utils import run_bass_kernel_spmd

F32 = mybir.dt.float32
BF16 = mybir.dt.bfloat16
AF = mybir.ActivationFunctionType

P = 128
H = 96
W = 96
B_LOC = 2       # batches per core
WH = 48         # columns per w-half
WC = 4          # columns per chunk
NCH = WH // WC  # 12 chunks per half
NG = B_LOC * 2 * NCH  # 48 chunks total per core


def _build():
    nc = bacc.Bacc("TRN2", target_bir_lowering=False, debug=False)

    # inputs: c-major x [b, half, ci, 128, 48*96], h-major xT [b, half, 96, 48*256]
    xc_d = nc.dram_tensor("xc", [B_LOC, 2, 2, P, WH * H], BF16, kind="ExternalInput")
    xt_d = nc.dram_tensor("xt", [B_LOC, 2, H, WH * 256], BF16, kind="ExternalInput")
    cb_d = nc.dram_tensor("cblob", [P, 1026], BF16, kind="ExternalInput")
    bb_d = nc.dram_tensor("bblob", [P, 2], F32, kind="ExternalInput")
    out_d = nc.dram_tensor("out", [B_LOC, 2, H, WH * 256], BF16, kind="ExternalOutput")

    with tile.TileContext(nc) as tc:
        import contextlib

        ctx = contextlib.ExitStack()
        with ctx:
            consts = ctx.enter_context(tc.tile_pool(name="consts", bufs=1))
            xcp = ctx.enter_context(tc.tile_pool(name="xcp", bufs=2))
            xtp = ctx.enter_context(tc.tile_pool(name="xtp", bufs=3))
            qkp = ctx.enter_context(tc.tile_pool(name="qkp", bufs=3))
            esp = ctx.enter_context(tc.tile_pool(name="esp", bufs=3))
            vtp = ctx.enter_context(tc.tile_pool(name="vtp", bufs=4))
            rp = ctx.enter_context(tc.tile_pool(name="rp", bufs=3))
            psq = ctx.enter_context(tc.tile_pool(name="psq", bufs=1, space="PSUM"))
            psk = ctx.enter_context(tc.tile_pool(name="psk", bufs=1, space="PSUM"))
            pss = ctx.enter_context(tc.tile_pool(name="pss", bufs=2, space="PSUM"))
            psv = ctx.enter_context(tc.tile_pool(name="psv", bufs=2, space="PSUM"))
            psu = ctx.enter_context(tc.tile_pool(name="psu", bufs=2, space="PSUM"))

            cb_t = consts.tile([P, 1026], BF16)
            bb_t = consts.tile([P, 2], F32)
            nc.sync.dma_start(out=cb_t, in_=cb_d.ap())
            nc.sync.dma_start(out=bb_t, in_=bb_d.ap())
            # observers: funnel const-DMA deps into single engine sems
            nc.tensor.ldweights(cb_t[:, 0:64])
            bias_t = consts.tile([P, 2], F32)
            nc.vector.tensor_copy(bias_t, bb_t)
            # q/k proj weights duplicated over both partition halves (M=128)
            wq_t = cb_t[:, 0:256].rearrange("p (c m) -> p c m", c=2)     # [128,2,128]
            wk_t = cb_t[:, 256:512].rearrange("p (c m) -> p c m", c=2)   # [128,2,128]
            wvt_t = cb_t[:, 512:1024].rearrange("p (c m) -> p c m", c=2)  # [128,2,256]
            bq_t = bias_t[:, 0:1]
            bk_t = bias_t[:, 1:2]
            invg_t = cb_t[0:H, 1025:1026]

            # per-chunk state passed across pipeline stages
            xc_tiles = {}   # (b, half) -> tile [128, 2, WH*H]
            xt_tiles = {}   # (b, half) -> tile [H, WH*256]
            qs = {}
            ks = {}
            ess = {}
            vts = {}
            pss_t = {}
            psu_t = {}
            rs = {}

            def bh(g):
                b, r = divmod(g, 2 * NCH)
                half, ch = divmod(r, NCH)
                return b, half, ch

            def load_bh(b, half, pieces=1):
                # pieces>1 splits each transfer so early chunks' slice deps
                # clear sooner (used for the very first load); piece 0 of each
                # stream is dispatched first so chunk 0 unblocks ASAP.
                # Steady-state prefetches go via the idle gpsimd queue.
                eng = nc.sync
                x_t = xcp.tile([P, 2, WH * H], BF16, tag="xc")
                t_t = xtp.tile([H, WH * 256], BF16, tag="xt")
                np_ = WH * H // pieces
                nt = WH * 256 // pieces
                for pc in range(pieces):
                    for ci in range(2):
                        eng.dma_start(
                            out=x_t[:, ci, pc * np_ : (pc + 1) * np_],
                            in_=xc_d.ap()[b, half, ci, :, pc * np_ : (pc + 1) * np_])
                    eng.dma_start(
                        out=t_t[:, pc * nt : (pc + 1) * nt],
                        in_=xt_d.ap()[b, half, :, pc * nt : (pc + 1) * nt])
                xc_tiles[(b, half)] = x_t
                xt_tiles[(b, half)] = t_t

            def st_proj(g):
                b, half, ch = bh(g)
                if ch == 0 and (b, half) not in xc_tiles:
                    load_bh(b, half, pieces=4 if g == 0 else 1)
                # prefetch next half's inputs one chunk into this half
                if ch == 1:
                    nb, nr = divmod(g + NCH, 2 * NCH)
                    nhalf = nr // NCH
                    if nb < B_LOC and (nb, nhalf) not in xc_tiles:
                        load_bh(nb, nhalf)
                x_t = xc_tiles[(b, half)]
                q_p = psq.tile([P, WC * H], F32, tag="q")
                k_p = psk.tile([P, WC * H], F32, tag="k")
                for ci in range(2):
                    rhs = x_t[:, ci, ch * WC * H : (ch + 1) * WC * H]
                    nc.tensor.matmul(q_p, wq_t[:, ci, :], rhs,
                                     start=(ci == 0), stop=(ci == 1))
                for ci in range(2):
                    rhs = x_t[:, ci, ch * WC * H : (ch + 1) * WC * H]
                    nc.tensor.matmul(k_p, wk_t[:, ci, :], rhs,
                                     start=(ci == 0), stop=(ci == 1))
                # evacs: k on ACT, q on DVE (q/k live duplicated in both halves)
                q_t = qkp.tile([P, WC * H], BF16, tag="qs")
                k_t = qkp.tile([P, WC * H], BF16, tag="ks")
                nc.scalar.activation(out=k_t, in_=k_p, func=AF.Identity, bias=bk_t)
                nc.vector.tensor_scalar(out=q_t, in0=q_p, scalar1=bq_t, scalar2=None,
                                        op0=mybir.AluOpType.add)
                qs[g], ks[g] = q_t, k_t

            def st_s_vt(g):
                b, half, ch = bh(g)
                x_t = xc_tiles[(b, half)]
                xcols = x_t.rearrange("p c (w h) -> p c w h", h=H)
                q_t, k_t = qs.pop(g), ks.pop(g)
                s_p = pss.tile([H, WC * H + WC], F32, tag="s")
                for j in range(WC):
                    # alternate PE row-groups (K=64): even cols read the 0:64
                    # copy, odd cols the 64:128 copy -> T0/T8 concurrency
                    base = 0  # bisect: base-64 path caused device fault
                    nc.tensor.matmul(
                        s_p[:, j * H : (j + 1) * H],
                        k_t[base : base + 64, j * H : (j + 1) * H],
                        q_t[base : base + 64, j * H : (j + 1) * H],
                        start=True, stop=True)
                es_t = esp.tile([H, WC * H], BF16, tag="es")
                nc.scalar.activation(out=es_t, in_=s_p[:, 0 : WC * H], func=AF.Exp)
                ess[g] = es_t
                pss_t[g] = s_p
                # V^T per column pair (independent of S chain)
                vt_pair = []
                for pair in range(2):
                    v_p = psv.tile([H, 512], F32, tag="v")
                    for j2 in range(2):
                        wl = ch * WC + pair * 2 + j2
                        for ci in range(2):
                            nc.tensor.matmul(
                                v_p[:, j2 * 256 : (j2 + 1) * 256],
                                xcols[:, ci, wl, :],
                                wvt_t[:, ci, :],
                                start=(ci == 0), stop=(ci == 1))
                    vt_t = vtp.tile([H, 512], BF16, tag="vt")
                    nc.scalar.copy(out=vt_t, in_=v_p)
                    vt_pair.append(vt_t)
                vts[g] = vt_pair

            def st_u(g):
                es_t = ess.pop(g)
                s_p = pss_t.pop(g)
                vt_pair = vts.pop(g)
                # colsum + recip first: shortens the es->recip->STT chain
                for j in range(WC):
                    nc.tensor.matmul(
                        s_p[:, WC * H + j : WC * H + j + 1],
                        es_t[:, j * H : (j + 1) * H],
                        invg_t,
                        start=True, stop=True)
                r_t = rp.tile([H, WC], F32, tag="r")
                nc.vector.reciprocal(out=r_t, in_=s_p[:, WC * H : WC * H + WC])
                u_ps = []
                for pair in range(2):
                    u_p = psu.tile([H, 512], F32, tag="u")
                    for j2 in range(2):
                        j = pair * 2 + j2
                        nc.tensor.matmul(
                            u_p[:, j2 * 256 : (j2 + 1) * 256],
                            es_t[:, j * H : (j + 1) * H],
                            vt_pair[pair][:, j2 * 256 : (j2 + 1) * 256],
                            start=True, stop=True)
                    u_ps.append(u_p)
                rs[g] = (u_ps, r_t)

            def st_fin(g):
                b, half, ch = bh(g)
                u_ps, r_t = rs.pop(g)
                t_t = xt_tiles[(b, half)]
                tv = t_t.rearrange("p (w c) -> p w c", c=256)
                for j in range(WC):
                    u_p = u_ps[j // 2]
                    u_slice = u_p[:, (j % 2) * 256 : (j % 2 + 1) * 256]
                    dst = tv[:, ch * WC + j, :]
                    nc.vector.scalar_tensor_tensor(
                        out=dst, in0=u_slice, scalar=r_t[:, j : j + 1], in1=dst,
                        op0=mybir.AluOpType.mult, op1=mybir.AluOpType.add)
                if ch % 4 == 3:
                    p3 = (WH // 3) * 256
                    pc = ch // 4
                    nc.sync.dma_start(
                        out=out_d.ap()[b, half, :, pc * p3 : (pc + 1) * p3],
                        in_=t_t[:, pc * p3 : (pc + 1) * p3])
                    if ch == NCH - 1:
                        del xc_tiles[(b, half)], xt_tiles[(b, half)]

            # software pipeline, depth 2
            for g in range(NG + 2):
                if g < NG:
                    st_proj(g)
                if 1 <= g < NG + 1:
                    st_s_vt(g - 1)
                if g >= 2:
                    st_u(g - 2)
                    st_fin(g - 2)
    nc.compile()
    return nc


_NC_CACHE = None


def _get_nc():
    global _NC_CACHE
    if _NC_CACHE is None:
        _NC_CACHE = _build()
    return _NC_CACHE


def _prep(x, Wq, bq, Wk, bk, Wv, bv, gamma):
    x = np.asarray(x, np.float32)
    Wq = np.asarray(Wq, np.float32)
    bq = np.asarray(bq, np.float32)
    Wk = np.asarray(Wk, np.float32)
    bk = np.asarray(bk, np.float32)
    Wv = np.asarray(Wv, np.float32)
    bv = np.asarray(bv, np.float32)
    g = float(np.asarray(gamma, np.float32)[0])

    C = 256
    e = (g * np.linalg.solve(np.eye(C, dtype=np.float64) + g * Wv.astype(np.float64),
                             bv.astype(np.float64))).astype(np.float32)
    xb = x + e[None, :, None, None]
    # xc: [16, half, ci, 128, 48, 96]  from (b, c, h, w) -> (b, c, w, h)
    xwh = np.ascontiguousarray(np.transpose(xb, (0, 1, 3, 2)))  # b, c, w, h
    xc = xwh.reshape(16, 2, P, 2, WH, H).transpose(0, 3, 1, 2, 4, 5)
    xc = np.ascontiguousarray(xc).astype(ml_dtypes.bfloat16)
    xc = xc.reshape(16, 2, 2, P, WH * H)
    # xt: [16, half, 96, 48, 256] from (b, h, w, c)
    xhwc = np.ascontiguousarray(np.transpose(xb, (0, 2, 3, 1)))  # b, h, w, c
    xt = xhwc.reshape(16, H, 2, WH, C).transpose(0, 2, 1, 3, 4)
    xt = np.ascontiguousarray(xt).astype(ml_dtypes.bfloat16)
    xt = xt.reshape(16, 2, H, WH * C)

    # blob: 0:256 wq(dup M=128), 256:512 wk(dup), 512:1024 wvt, col 1025 invg
    blob = np.zeros((P, 1026), np.float32)
    wqd = [np.concatenate([Wq[:, s].T, Wq[:, s].T], axis=1)
           for s in (slice(0, 128), slice(128, 256))]  # each [128, 128]
    wkd = [np.concatenate([Wk[:, s].T, Wk[:, s].T], axis=1)
           for s in (slice(0, 128), slice(128, 256))]
    blob[:, 0:256] = np.stack(wqd, axis=1).reshape(P, 256)
    blob[:, 256:512] = np.stack(wkd, axis=1).reshape(P, 256)
    blob[:, 512:1024] = np.stack([Wv.T[:128], Wv.T[128:]], axis=1).reshape(P, 512)
    blob[0:H, 1025] = 1.0 / g
    blob = blob.astype(ml_dtypes.bfloat16)

    bqe = bq - Wq @ e
    bke = bk - Wk @ e
    bblob = np.zeros((P, 2), np.float32)
    bblob[0:64, 0] = bqe
    bblob[64:128, 0] = bqe
    bblob[0:64, 1] = bke
    bblob[64:128, 1] = bke
    return xc, xt, blob, bblob


def kernel(x, Wq, bq, Wk, bk, Wv, bv, gamma):
    xc, xt, blob, bblob = _prep(x, Wq, bq, Wk, bk, Wv, bv, gamma)
    nc = _get_nc()
    in_maps = []
    for core in range(8):
        in_maps.append({
            "xc": xc[core * B_LOC : (core + 1) * B_LOC],
            "xt": xt[core * B_LOC : (core + 1) * B_LOC],
            "cblob": blob, "bblob": bblob,
        })
    res = run_bass_kernel_spmd(nc, in_maps, core_ids=list(range(8)))
    outs = [r["out"] for r in res.results]
    full = np.concatenate(outs, axis=0)  # [16, 2, 96, 48*256] bf16
    full = full.reshape(16, 2, H, WH, 256).astype(np.float32)
    # (b, half, h, w48, c) -> (b, c, h, w)
    full = full.transpose(0, 4, 2, 1, 3).reshape(16, 256, H, W)
    return np.ascontiguousarray(full)


def prepared_in_maps(inputs):
    xc, xt, blob, bblob = _prep(**inputs)
    return [
        {"xc": xc[c * B_LOC : (c + 1) * B_LOC], "xt": xt[c * B_LOC : (c + 1) * B_LOC],
         "cblob": blob, "bblob": bblob}
        for c in range(8)
    ]

